# revision 1
# baseline (speedup 1.0000x reference)
"""Trainium2 Bass kernel for a 6-layer post-LN Transformer encoder.

Strategy (8 NeuronCores):
  - Sequence-parallel: cores 0-3 own batch 0, cores 4-7 own batch 1; each core
    owns 512 tokens. Weights are replicated (bf16). Per layer, each 4-core
    group AllGathers K^T and V (bf16) so every core attends over the full
    2048-token sequence with its local 512 queries.
  - Activations are kept feature-major ([D, tok]) on-chip, so every matmul
    consumes operands in natural layout and no transposes are needed.
    LayerNorm statistics are computed with ones-vector matmuls on the
    TensorEngine (partition-direction reduction).
  - Softmax: scores are computed transposed (s^T[k, q]) with 2 heads packed
    per matmul via tile_position row-tiling (dk=64). exp() runs on the
    Scalar engine with the mask bias fused per-partition; the denominator
    comes for free from a ones-column appended to V in the attn@V matmul.
"""

import numpy as np
import ml_dtypes

L, D, H, FF = 6, 1024, 16, 4096
DK = D // H          # 64
B, S = 2, 2048
NCORES = 8
R = 4                # cores per batch group
T = S // R           # 512 local tokens per core
DC = D // 128        # 8
FC = FF // 128       # 32
KC = S // 128        # 16
TC = T // 128        # 4
EPS = 1e-5
BF16 = ml_dtypes.bfloat16

_CACHE = {}


def _build_nc():
    import contextlib
    import concourse.bacc as bacc
    import concourse.mybir as mybir
    import concourse.tile as tile
    import concourse.bass as bass
    from concourse.bass import ts, ds

    f32 = mybir.dt.float32
    bf16 = mybir.dt.bfloat16
    AF = mybir.ActivationFunctionType
    OP = mybir.AluOpType

    nc = bacc.Bacc(num_devices=NCORES)

    # ---- parameters -----------------------------------------------------
    x0T = nc.declare_dram_parameter("x0T", [D, T], f32, isOutput=False)
    maskb = nc.declare_dram_parameter("maskb", [128, KC], f32, isOutput=False)
    wq = nc.declare_dram_parameter("wq", [L, D, D], bf16, isOutput=False)
    wk = nc.declare_dram_parameter("wk", [L, D, D], bf16, isOutput=False)
    wv = nc.declare_dram_parameter("wv", [L, D, D], bf16, isOutput=False)
    wo = nc.declare_dram_parameter("wo", [L, D, D], bf16, isOutput=False)
    w1 = nc.declare_dram_parameter("w1", [L, D, FF], bf16, isOutput=False)
    # W2 pre-packed on host: [L, mc(8), o(32), p(128), m(128)]
    w2p = nc.declare_dram_parameter("w2p", [L, DC, FC, 128, 128], bf16, isOutput=False)
    bq = nc.declare_dram_parameter("bq", [L, D], f32, isOutput=False)
    bk = nc.declare_dram_parameter("bk", [L, D], f32, isOutput=False)
    bvb = nc.declare_dram_parameter("bvb", [L, D], bf16, isOutput=False)
    bo = nc.declare_dram_parameter("bo", [L, D], f32, isOutput=False)
    b1 = nc.declare_dram_parameter("b1", [L, FF], f32, isOutput=False)
    b2 = nc.declare_dram_parameter("b2", [L, D], f32, isOutput=False)
    g1 = nc.declare_dram_parameter("g1", [L, D], f32, isOutput=False)
    be1 = nc.declare_dram_parameter("be1", [L, D], f32, isOutput=False)
    g2 = nc.declare_dram_parameter("g2", [L, D], f32, isOutput=False)
    be2 = nc.declare_dram_parameter("be2", [L, D], f32, isOutput=False)
    outT = nc.declare_dram_parameter("outT", [D, T], f32, isOutput=True)

    groups = [[0, 1, 2, 3], [4, 5, 6, 7]]

    with tile.TileContext(nc) as tc:
        ctx = contextlib.ExitStack()
        singles = ctx.enter_context(tc.tile_pool(name="singles", bufs=1))
        params = ctx.enter_context(tc.tile_pool(name="params", bufs=2))
        wpool = ctx.enter_context(tc.tile_pool(name="wpool", bufs=2))
        w2pool = ctx.enter_context(tc.tile_pool(name="w2pool", bufs=2))
        kgpool = ctx.enter_context(tc.tile_pool(name="kgpool", bufs=2))
        vgpool = ctx.enter_context(tc.tile_pool(name="vgpool", bufs=1))
        apool = ctx.enter_context(tc.tile_pool(name="apool", bufs=8))
        xbfpool = ctx.enter_context(tc.tile_pool(name="xbfpool", bufs=2))
        qkvpool = ctx.enter_context(tc.tile_pool(name="qkvpool", bufs=2))
        anfpool = ctx.enter_context(tc.tile_pool(name="anfpool", bufs=8))
        ao65pool = ctx.enter_context(tc.tile_pool(name="ao65pool", bufs=2))
        tmp = ctx.enter_context(tc.tile_pool(name="tmp", bufs=3))
        small = ctx.enter_context(tc.tile_pool(name="small", bufs=1))
        denp = ctx.enter_context(tc.tile_pool(name="denp", bufs=2))
        rbp = ctx.enter_context(tc.tile_pool(name="rbp", bufs=1))
        dram = ctx.enter_context(tc.tile_pool(name="dram", bufs=2, space="DRAM"))
        pscore = ctx.enter_context(tc.tile_pool(name="pscore", bufs=2, space="PSUM"))
        pav = ctx.enter_context(tc.tile_pool(name="pav", bufs=2, space="PSUM"))
        pmisc = ctx.enter_context(tc.tile_pool(name="pmisc", bufs=2, space="PSUM"))

        # ---- constants + resident state --------------------------------
        xT = singles.tile([128, DC, T], f32, name="xT")
        nc.sync.dma_start(out=xT, in_=x0T[:, :].rearrange("(c p) t -> p c t", p=128))
        mb_sb = singles.tile([128, KC], f32, name="mb_sb")
        nc.sync.dma_start(out=mb_sb, in_=maskb[:, :])
        ones_col = singles.tile([128, 1], f32, name="ones_col")
        nc.vector.memset(ones_col, 1.0)
        ones_row = singles.tile([1, 128], f32, name="ones_row")
        nc.vector.memset(ones_row, 1.0)
        ones_row_bf = singles.tile([1, 128], bf16, name="ones_row_bf")
        nc.vector.memset(ones_row_bf, 1.0)
        eps_sb = singles.tile([1, 1], f32, name="eps_sb")
        nc.vector.memset(eps_sb, EPS)
        xbf_cur = xbfpool.tile([128, DC, T], bf16, tag="xbf")
        for c in range(DC):
            nc.vector.tensor_copy(xbf_cur[:, c, :], xT[:, c, :])

        def layernorm_inplace(g_sb, be_sb, gi, xbf_out):
            """x = LN(x) in place; also writes bf16 copy into xbf_out."""
            psum_sum = pmisc.tile([128, 512], f32, tag="pmisc")
            for c in range(DC):
                nc.tensor.matmul(psum_sum[0:1, :], ones_col, xT[:, c, :],
                                 start=(c == 0), stop=(c == DC - 1))
            psum_sq = pmisc.tile([128, 512], f32, tag="pmisc")
            for c in range(DC):
                sq = tmp.tile([128, 512], f32, tag="tmp")
                nc.vector.tensor_mul(sq, xT[:, c, :], xT[:, c, :])
                nc.tensor.matmul(psum_sq[0:1, :], ones_col, sq,
                                 start=(c == 0), stop=(c == DC - 1))
            mr = small.tile([1, 1024], f32, tag="mr")
            e2 = small.tile([1, 512], f32, tag="e2")
            msq = small.tile([1, 512], f32, tag="msq")
            nc.scalar.mul(mr[:, 0:512], psum_sum[0:1, :], 1.0 / D)
            nc.scalar.mul(e2, psum_sq[0:1, :], 1.0 / D)
            nc.vector.tensor_mul(msq, mr[:, 0:512], mr[:, 0:512])
            nc.vector.tensor_tensor(e2, e2, msq, OP.subtract)
            lnv = small.tile([1, 512], f32, tag="lnv")
            nc.scalar.activation(lnv, e2, AF.Ln, bias=eps_sb)
            nc.scalar.activation(mr[:, 512:1024], lnv, AF.Exp, scale=-0.5)
            bc = pscore.tile([128, 1024], f32, tag="pscore")
            nc.tensor.matmul(bc[:, 0:512], ones_row, mr[:, 0:512],
                             start=True, stop=True)
            nc.tensor.matmul(bc[:, 512:1024], ones_row, mr[:, 512:1024],
                             start=True, stop=True)
            for c in range(DC):
                t1 = tmp.tile([128, 512], f32, tag="tmp")
                nc.vector.tensor_tensor(t1, xT[:, c, :], bc[:, 0:512], OP.subtract)
                nc.vector.tensor_tensor(t1, t1, bc[:, 512:1024], OP.mult)
                nc.vector.tensor_scalar(xT[:, c, :], t1, g_sb[:, c:c + 1],
                                        be_sb[:, c:c + 1], OP.mult, OP.add)
                nc.vector.tensor_copy(xbf_out[:, c, :], xT[:, c, :])

        for l in range(L):
            # ---- per-layer params --------------------------------------
            pp = params.tile([128, 8, DC], f32, tag="pcol")
            for i_, t_src in enumerate([bq, bk, bo, b2, g1, be1, g2, be2]):
                nc.sync.dma_start(out=pp[:, i_, :],
                                  in_=t_src[l].rearrange("(c p) -> p c", p=128))
            bq_sb, bk_sb, bo_sb, b2_sb = pp[:, 0], pp[:, 1], pp[:, 2], pp[:, 3]
            g1_sb, be1_sb, g2_sb, be2_sb = pp[:, 4], pp[:, 5], pp[:, 6], pp[:, 7]
            b1_sb = params.tile([128, FC], f32, tag="pc32")
            nc.sync.dma_start(out=b1_sb, in_=b1[l].rearrange("(c p) -> p c", p=128))
            bv_row = params.tile([1, D], bf16, tag="bv_row")
            nc.sync.dma_start(out=bv_row, in_=bvb[l][None, :])

            xbf = xbf_cur

            # ---- K projection + AllGather ------------------------------
            kt_loc = dram.tile([D, T], bf16, tag="kt_loc")
            kt_g1 = dram.tile([R * D // 2, T], bf16, tag="kt_g1")
            kt_g2 = dram.tile([R * D // 2, T], bf16, tag="kt_g2")
            wk_sb = wpool.tile([128, DC, D], bf16, tag="w")
            nc.sync.dma_start(out=wk_sb, in_=wk[l].rearrange("(c p) m -> p c m", p=128))
            kt_sb = qkvpool.tile([128, DC, T], bf16, tag="qkv")
            for half in range(2):
                for mc in range(half * 4, half * 4 + 4):
                    ps = pmisc.tile([128, 512], f32, tag="pmisc")
                    for c in range(DC):
                        nc.tensor.matmul(ps, wk_sb[:, c, ts(mc, 128)], xbf[:, c, :],
                                         start=(c == 0), stop=(c == DC - 1))
                    nc.vector.tensor_scalar(kt_sb[:, mc, :], ps,
                                            bk_sb[:, mc:mc + 1], None, OP.add)
                nc.gpsimd.dma_start(
                    out=kt_loc[ds(half * D // 2, D // 2), :]
                        .rearrange("(c p) t -> p c t", p=128),
                    in_=kt_sb[:, ds(half * 4, 4), :])
                nc.gpsimd.collective_compute(
                    "AllGather", OP.bypass, replica_groups=groups,
                    ins=[kt_loc[ds(half * D // 2, D // 2), :].opt()],
                    outs=[(kt_g1 if half == 0 else kt_g2).opt()])

            # ---- V projection + AllGather ------------------------------
            v_loc = dram.tile([T, D], bf16, tag="v_loc")
            v_g = dram.tile([R * T, D], bf16, tag="v_g")
            wv_sb = wpool.tile([128, DC, D], bf16, tag="w")
            nc.sync.dma_start(out=wv_sb, in_=wv[l].rearrange("(c p) m -> p c m", p=128))
            v_sb = qkvpool.tile([128, TC, D], bf16, tag="qkv")
            for t_ in range(TC):
                for nh in range(2):
                    ps = pmisc.tile([128, 512], f32, tag="pmisc")
                    for c in range(DC):
                        nc.tensor.matmul(ps, xbf[:, c, ts(t_, 128)],
                                         wv_sb[:, c, ds(nh * 512, 512)],
                                         start=(c == 0), stop=False)
                    nc.tensor.matmul(ps, ones_row_bf, bv_row[:, ds(nh * 512, 512)],
                                     start=False, stop=True)
                    nc.vector.tensor_copy(v_sb[:, t_, ds(nh * 512, 512)], ps)
            nc.gpsimd.dma_start(out=v_loc[:, :].rearrange("(c p) d -> p c d", p=128),
                                in_=v_sb)
            nc.gpsimd.collective_compute(
                "AllGather", OP.bypass, replica_groups=groups,
                ins=[v_loc.opt()], outs=[v_g.opt()])

            # ---- Q projection ------------------------------------------
            wq_sb = wpool.tile([128, DC, D], bf16, tag="w")
            nc.sync.dma_start(out=wq_sb, in_=wq[l].rearrange("(c p) m -> p c m", p=128))
            qT = qkvpool.tile([128, DC, T], bf16, tag="qkv")
            for mc in range(DC):
                ps = pmisc.tile([128, 512], f32, tag="pmisc")
                for c in range(DC):
                    nc.tensor.matmul(ps, wq_sb[:, c, ts(mc, 128)], xbf[:, c, :],
                                     start=(c == 0), stop=(c == DC - 1))
                nc.vector.tensor_scalar(qT[:, mc, :], ps,
                                        bq_sb[:, mc:mc + 1], None, OP.add)

            # ---- gathered V -> SBUF with ones column -------------------
            vg_sb = vgpool.tile([128, KC, H * 65], bf16, tag="vg")
            for kc in range(KC):
                dst = vg_sb[:, kc, :].rearrange("p (h w) -> p h w", w=65)
                nc.sync.dma_start(
                    out=dst[:, :, 0:64],
                    in_=v_g[ds(kc * 128, 128), :].rearrange("p (h w) -> p h w", w=64))
                nc.vector.memset(dst[:, :, 64:65], 1.0)

            # ---- attention ---------------------------------------------
            den_sb = denp.tile([16, 512], f32, tag="den")
            an_bf = xbfpool.tile([128, DC, T], bf16, tag="xbf")
            anf_tiles = []
            for j in range(DC):  # head pairs (2j, 2j+1)
                kgj = kgpool.tile([128, R, T], bf16, tag="kg")
                kt_gh = kt_g1 if j < 4 else kt_g2
                nc.sync.dma_start(
                    out=kgj,
                    in_=kt_gh[:, :].rearrange("(r c p) t -> p r c t", p=128,
                                              c=4)[:, :, j % 4, :])
                at_tiles = []
                for kc in range(KC):
                    r, c4 = kc // 4, kc % 4
                    pss = pscore.tile([128, 1024], f32, tag="pscore")
                    nc.tensor.matmul(pss[:, 0:512], kgj[0:64, r, ts(c4, 128)],
                                     qT[0:64, j, :], start=True, stop=True,
                                     tile_position=(0, 0))
                    nc.tensor.matmul(pss[:, 512:1024], kgj[64:128, r, ts(c4, 128)],
                                     qT[64:128, j, :], start=True, stop=True,
                                     tile_position=(64, 0))
                    at = apool.tile([128, 1024], bf16, tag="attn")
                    nc.scalar.activation(at, pss, AF.Exp, scale=1.0 / 32.0,
                                         bias=mb_sb[:, kc:kc + 1])
                    at_tiles.append(at)
                anf = anfpool.tile([128, 512], f32, tag="anf")
                for ab in range(2):
                    h = 2 * j + ab
                    pav_t = pav.tile([65, 512], f32, tag="pav")
                    for kc in range(KC):
                        nc.tensor.matmul(pav_t,
                                         vg_sb[:, kc, ds(h * 65, 65)],
                                         at_tiles[kc][:, ds(ab * 512, 512)],
                                         start=(kc == 0), stop=(kc == KC - 1))
                    # evict to SBUF (same partitions), then DMA to re-pack
                    ao65 = ao65pool.tile([65, 512], f32, tag="ao65")
                    nc.vector.tensor_copy(ao65, pav_t)
                    nc.sync.dma_start(out=den_sb[h:h + 1, :], in_=ao65[64:65, :])
                    nc.sync.dma_start(out=anf[ds(ab * 64, 64), :], in_=ao65[0:64, :])
                anf_tiles.append(anf)

            # ---- normalize + pack attn_out -----------------------------
            den_r = denp.tile([16, 512], f32, tag="den")
            nc.vector.reciprocal(den_r, den_sb)
            denr_d = dram.tile([16, 512], f32, tag="denr")
            nc.sync.dma_start(out=denr_d[:, :], in_=den_r)
            for j in range(DC):
                rb = rbp.tile([128, 512], f32, tag="rb")
                for ab in range(2):
                    src = bass.AP(tensor=denr_d.tensor,
                                  offset=denr_d.offset + (2 * j + ab) * 512,
                                  ap=[[0, 64], [1, 512]])
                    nc.sync.dma_start(out=rb[ds(ab * 64, 64), :], in_=src)
                nc.vector.tensor_tensor(an_bf[:, j, :], anf_tiles[j], rb, OP.mult)

            # ---- Wo + residual -----------------------------------------
            wo_sb = wpool.tile([128, DC, D], bf16, tag="w")
            nc.sync.dma_start(out=wo_sb, in_=wo[l].rearrange("(c p) m -> p c m", p=128))
            for mc in range(DC):
                ps = pmisc.tile([128, 512], f32, tag="pmisc")
                for c in range(DC):
                    nc.tensor.matmul(ps, wo_sb[:, c, ts(mc, 128)], an_bf[:, c, :],
                                     start=(c == 0), stop=(c == DC - 1))
                nc.vector.scalar_tensor_tensor(xT[:, mc, :], ps,
                                               bo_sb[:, mc:mc + 1], xT[:, mc, :],
                                               OP.add, OP.add)

            # ---- LN1 ----------------------------------------------------
            xbf2 = xbfpool.tile([128, DC, T], bf16, tag="xbf")
            layernorm_inplace(g1_sb, be1_sb, 1, xbf2)

            # ---- FFN ----------------------------------------------------
            ffT = vgpool.tile([128, FC, T], bf16, tag="vg")
            for b4 in range(4):
                w1b = wpool.tile([128, DC, 1024], bf16, tag="w")
                nc.sync.dma_start(
                    out=w1b,
                    in_=w1[l][:, ds(b4 * 1024, 1024)].rearrange("(c p) m -> p c m", p=128))
                for mcb in range(DC):
                    mc = b4 * 8 + mcb
                    ps = pmisc.tile([128, 512], f32, tag="pmisc")
                    for c in range(DC):
                        nc.tensor.matmul(ps, w1b[:, c, ts(mcb, 128)], xbf2[:, c, :],
                                         start=(c == 0), stop=(c == DC - 1))
                    nc.vector.tensor_scalar(ffT[:, mc, :], ps,
                                            b1_sb[:, mc:mc + 1], 0.0,
                                            OP.add, OP.max)
            for mc in range(DC):
                w2mc = w2pool.tile([128, FC, 128], bf16, tag="w2")
                nc.sync.dma_start(out=w2mc,
                                  in_=w2p[l, mc].rearrange("o p m -> p o m"))
                ps = pmisc.tile([128, 512], f32, tag="pmisc")
                for fc in range(FC):
                    nc.tensor.matmul(ps, w2mc[:, fc, :], ffT[:, fc, :],
                                     start=(fc == 0), stop=(fc == FC - 1))
                nc.vector.scalar_tensor_tensor(xT[:, mc, :], ps,
                                               b2_sb[:, mc:mc + 1], xT[:, mc, :],
                                               OP.add, OP.add)

            # ---- LN2 ----------------------------------------------------
            xbf_cur = xbfpool.tile([128, DC, T], bf16, tag="xbf")
            layernorm_inplace(g2_sb, be2_sb, 2, xbf_cur)

        # ---- output ----------------------------------------------------
        nc.sync.dma_start(out=outT[:, :].rearrange("(c p) t -> p c t", p=128), in_=xT)
        ctx.close()

    nc.compile()
    return nc


def _prepare_host(inputs):
    src = np.asarray(inputs["src"]).astype(np.int64)
    emb = np.asarray(inputs["emb"], dtype=np.float32)
    x = emb[src]                                    # [B, S, D] f32
    pos = np.arange(B, dtype=np.float32)[:, None]
    div = np.exp(np.arange(0, D, 2, dtype=np.float32) * (-np.log(10000.0) / D))
    pe = np.zeros((B, D), np.float32)
    pe[:, 0::2] = np.sin(pos / div)
    pe[:, 1::2] = np.cos(pos / div)
    x = x + pe[:, None, :]

    mask = np.asarray(inputs["src_mask"]).reshape(B, S)
    mbias = np.where(mask != 0, 0.0, -1e9).astype(np.float32)   # [B, S]

    f = np.float32
    shared = {
        "wq": np.ascontiguousarray(np.asarray(inputs["Wq"], f).astype(BF16)),
        "wk": np.ascontiguousarray(np.asarray(inputs["Wk"], f).astype(BF16)),
        "wv": np.ascontiguousarray(np.asarray(inputs["Wv"], f).astype(BF16)),
        "wo": np.ascontiguousarray(np.asarray(inputs["Wo"], f).astype(BF16)),
        "w1": np.ascontiguousarray(np.asarray(inputs["W1"], f).astype(BF16)),
        "w2p": np.ascontiguousarray(
            np.asarray(inputs["W2"], f).reshape(L, FC, 128, DC, 128)
            .transpose(0, 3, 1, 2, 4).astype(BF16)),
        "bq": np.ascontiguousarray(np.asarray(inputs["bq"], f)),
        "bk": np.ascontiguousarray(np.asarray(inputs["bk"], f)),
        "bvb": np.ascontiguousarray(np.asarray(inputs["bv"], f).astype(BF16)),
        "bo": np.ascontiguousarray(np.asarray(inputs["bo"], f)),
        "b1": np.ascontiguousarray(np.asarray(inputs["b1"], f)),
        "b2": np.ascontiguousarray(np.asarray(inputs["b2"], f)),
        "g1": np.ascontiguousarray(np.asarray(inputs["g1"], f)),
        "be1": np.ascontiguousarray(np.asarray(inputs["be1"], f)),
        "g2": np.ascontiguousarray(np.asarray(inputs["g2"], f)),
        "be2": np.ascontiguousarray(np.asarray(inputs["be2"], f)),
    }
    in_maps = []
    for i in range(NCORES):
        b = i // R
        t0 = (i % R) * T
        m = dict(shared)
        m["x0T"] = np.ascontiguousarray(x[b, t0:t0 + T, :].T.astype(np.float32))
        m["maskb"] = np.ascontiguousarray(mbias[b].reshape(KC, 128).T)
        in_maps.append(m)
    return in_maps


def _run(in_maps, trace=False):
    from concourse.bass_utils import run_bass_kernel_spmd
    if "nc" not in _CACHE:
        _CACHE["nc"] = _build_nc()
    nc = _CACHE["nc"]
    res = run_bass_kernel_spmd(nc, in_maps, core_ids=list(range(NCORES)),
                               trace=trace)
    outs = res.results
    y = np.zeros((B, S, D), np.float32)
    for i in range(NCORES):
        b = i // R
        t0 = (i % R) * T
        y[b, t0:t0 + T, :] = outs[i]["outT"].T
    return y, res


def kernel(**inputs) -> np.ndarray:
    in_maps = _prepare_host(inputs)
    y, _ = _run(in_maps, trace=False)
    return y


def kernel_traced(**inputs):
    """Same as kernel() but returns (output, BassKernelResults with profile)."""
    in_maps = _prepare_host(inputs)
    return _run(in_maps, trace=True)



# revision 7
# speedup vs baseline: 1.4171x; 1.4171x over previous
"""Trainium2 Bass kernel for a 6-layer post-LN Transformer encoder.

Strategy (8 NeuronCores):
  - Sequence-parallel: cores 0-3 own batch 0, cores 4-7 own batch 1; each core
    owns 512 tokens. Weights are replicated (bf16).
  - Attention is computed flash-style against the LOCAL key/value shard only
    (512 keys), producing an unnormalized attention output plus the softmax
    denominator (via a ones-column appended to V). A single bf16 AllReduce
    (split in two halves to overlap with compute) across the 4-core group sums
    the partials over key shards; normalization happens after the reduce.
    This removes the K/V AllGathers entirely and cuts per-core scores/exp/
    attn@V work 4x vs. gathered attention.
  - Activations are kept feature-major ([D, tok]) on-chip, so every matmul
    consumes operands in natural layout and no transposes are needed.
    LayerNorm statistics are computed with ones-vector matmuls on the
    TensorEngine (partition-direction reduction).
  - Softmax: scores are computed transposed (s^T[k, q]) with 2 heads packed
    per matmul via tile_position row-tiling (dk=64). exp() runs on the
    Scalar engine with the mask bias fused per-partition.
"""

import numpy as np
import ml_dtypes

L, D, H, FF = 6, 1024, 16, 4096
DK = D // H          # 64
B, S = 2, 2048
NCORES = 8
R = 4                # cores per batch group
T = S // R           # 512 local tokens per core
DC = D // 128        # 8
FC = FF // 128       # 32
TC = T // 128        # 4  (local key chunks)
EPS = 1e-5
BF16 = ml_dtypes.bfloat16

_CACHE = {}


def _build_nc():
    import contextlib
    import concourse.bacc as bacc
    import concourse.mybir as mybir
    import concourse.tile as tile
    import concourse.bass as bass
    from concourse.bass import ts, ds

    f32 = mybir.dt.float32
    bf16 = mybir.dt.bfloat16
    AF = mybir.ActivationFunctionType
    OP = mybir.AluOpType

    nc = bacc.Bacc(num_devices=NCORES)

    # ---- parameters -----------------------------------------------------
    x0T = nc.declare_dram_parameter("x0T", [D, T], f32, isOutput=False)
    maskb = nc.declare_dram_parameter("maskb", [128, TC], f32, isOutput=False)
    wq = nc.declare_dram_parameter("wq", [L, D, D], bf16, isOutput=False)
    wk = nc.declare_dram_parameter("wk", [L, D, D], bf16, isOutput=False)
    wv = nc.declare_dram_parameter("wv", [L, D, D], bf16, isOutput=False)
    wo = nc.declare_dram_parameter("wo", [L, D, D], bf16, isOutput=False)
    w1 = nc.declare_dram_parameter("w1", [L, D, FF], bf16, isOutput=False)
    # W2 pre-packed on host: [L, mc(8), o(32), p(128), m(128)]
    w2p = nc.declare_dram_parameter("w2p", [L, DC, FC, 128, 128], bf16, isOutput=False)
    bq = nc.declare_dram_parameter("bq", [L, D], f32, isOutput=False)
    bk = nc.declare_dram_parameter("bk", [L, D], f32, isOutput=False)
    bvb = nc.declare_dram_parameter("bvb", [L, D], bf16, isOutput=False)
    bo = nc.declare_dram_parameter("bo", [L, D], f32, isOutput=False)
    b1 = nc.declare_dram_parameter("b1", [L, FF], f32, isOutput=False)
    b2 = nc.declare_dram_parameter("b2", [L, D], f32, isOutput=False)
    g1 = nc.declare_dram_parameter("g1", [L, D], f32, isOutput=False)
    be1 = nc.declare_dram_parameter("be1", [L, D], f32, isOutput=False)
    g2 = nc.declare_dram_parameter("g2", [L, D], f32, isOutput=False)
    be2 = nc.declare_dram_parameter("be2", [L, D], f32, isOutput=False)
    outT = nc.declare_dram_parameter("outT", [D, T], f32, isOutput=True)

    groups = [[0, 1, 2, 3], [4, 5, 6, 7]]

    with tile.TileContext(nc) as tc:
        ctx = contextlib.ExitStack()
        singles = ctx.enter_context(tc.tile_pool(name="singles", bufs=1))
        params = ctx.enter_context(tc.tile_pool(name="params", bufs=2))
        wpool = ctx.enter_context(tc.tile_pool(name="wpool", bufs=2))
        w2pool = ctx.enter_context(tc.tile_pool(name="w2pool", bufs=2))
        ktpool = ctx.enter_context(tc.tile_pool(name="ktpool", bufs=2))
        qtpool = ctx.enter_context(tc.tile_pool(name="qtpool", bufs=2))
        vpool = ctx.enter_context(tc.tile_pool(name="vpool", bufs=2))
        apool = ctx.enter_context(tc.tile_pool(name="apool", bufs=8))
        xbfpool = ctx.enter_context(tc.tile_pool(name="xbfpool", bufs=2))
        anfpool = ctx.enter_context(tc.tile_pool(name="anfpool", bufs=4))
        ao65pool = ctx.enter_context(tc.tile_pool(name="ao65pool", bufs=2))
        tmp = ctx.enter_context(tc.tile_pool(name="tmp", bufs=2))
        small = ctx.enter_context(tc.tile_pool(name="small", bufs=1))
        denp = ctx.enter_context(tc.tile_pool(name="denp", bufs=2))
        rbp = ctx.enter_context(tc.tile_pool(name="rbp", bufs=2))
        dram = ctx.enter_context(tc.tile_pool(name="dram", bufs=2, space="DRAM"))
        pscore = ctx.enter_context(tc.tile_pool(name="pscore", bufs=2, space="PSUM"))
        pav = ctx.enter_context(tc.tile_pool(name="pav", bufs=2, space="PSUM"))
        pmisc = ctx.enter_context(tc.tile_pool(name="pmisc", bufs=2, space="PSUM"))

        # ---- constants + resident state --------------------------------
        xT = singles.tile([128, DC, T], f32, name="xT")
        nc.sync.dma_start(out=xT, in_=x0T[:, :].rearrange("(c p) t -> p c t", p=128))
        mb_sb = singles.tile([128, TC], f32, name="mb_sb")
        nc.sync.dma_start(out=mb_sb, in_=maskb[:, :])
        ones_col = singles.tile([128, 1], f32, name="ones_col")
        nc.vector.memset(ones_col, 1.0)
        ones_row = singles.tile([1, 128], f32, name="ones_row")
        nc.vector.memset(ones_row, 1.0)
        ones_row_bf = singles.tile([1, 128], bf16, name="ones_row_bf")
        nc.vector.memset(ones_row_bf, 1.0)
        eps_sb = singles.tile([1, 1], f32, name="eps_sb")
        nc.vector.memset(eps_sb, EPS)
        xbf_cur = xbfpool.tile([128, DC, T], bf16, tag="xbf")
        for c in range(DC):
            nc.vector.tensor_copy(xbf_cur[:, c, :], xT[:, c, :])

        def layernorm_inplace(g_sb, be_sb, gi, xbf_out):
            """x = LN(x) in place; also writes bf16 copy into xbf_out."""
            psum_sum = pmisc.tile([128, 512], f32, tag="pmisc")
            for c in range(DC):
                nc.tensor.matmul(psum_sum[0:1, :], ones_col, xT[:, c, :],
                                 start=(c == 0), stop=(c == DC - 1))
            psum_sq = pmisc.tile([128, 512], f32, tag="pmisc")
            for c in range(DC):
                sq = tmp.tile([128, 512], f32, tag="tmp")
                nc.vector.tensor_mul(sq, xT[:, c, :], xT[:, c, :])
                nc.tensor.matmul(psum_sq[0:1, :], ones_col, sq,
                                 start=(c == 0), stop=(c == DC - 1))
            mr = small.tile([1, 1024], f32, tag="mr")
            e2 = small.tile([1, 512], f32, tag="e2")
            msq = small.tile([1, 512], f32, tag="msq")
            nc.scalar.mul(mr[:, 0:512], psum_sum[0:1, :], 1.0 / D)
            nc.scalar.mul(e2, psum_sq[0:1, :], 1.0 / D)
            nc.vector.tensor_mul(msq, mr[:, 0:512], mr[:, 0:512])
            nc.vector.tensor_tensor(e2, e2, msq, OP.subtract)
            lnv = small.tile([1, 512], f32, tag="lnv")
            nc.scalar.activation(lnv, e2, AF.Ln, bias=eps_sb)
            nc.scalar.activation(mr[:, 512:1024], lnv, AF.Exp, scale=-0.5)
            bc = pscore.tile([128, 1024], f32, tag="pscore")
            nc.tensor.matmul(bc[:, 0:512], ones_row, mr[:, 0:512],
                             start=True, stop=True)
            nc.tensor.matmul(bc[:, 512:1024], ones_row, mr[:, 512:1024],
                             start=True, stop=True)
            for c in range(DC):
                t1 = tmp.tile([128, 512], f32, tag="tmp")
                nc.vector.tensor_tensor(t1, xT[:, c, :], bc[:, 0:512], OP.subtract)
                nc.vector.tensor_tensor(t1, t1, bc[:, 512:1024], OP.mult)
                nc.vector.tensor_scalar(xT[:, c, :], t1, g_sb[:, c:c + 1],
                                        be_sb[:, c:c + 1], OP.mult, OP.add)
                nc.vector.tensor_copy(xbf_out[:, c, :], xT[:, c, :])

        for l in range(L):
            # ---- per-layer params --------------------------------------
            pp = params.tile([128, 8, DC], f32, tag="pcol")
            for i_, t_src in enumerate([bq, bk, bo, b2, g1, be1, g2, be2]):
                nc.sync.dma_start(out=pp[:, i_, :],
                                  in_=t_src[l].rearrange("(c p) -> p c", p=128))
            bq_sb, bk_sb, bo_sb, b2_sb = pp[:, 0], pp[:, 1], pp[:, 2], pp[:, 3]
            g1_sb, be1_sb, g2_sb, be2_sb = pp[:, 4], pp[:, 5], pp[:, 6], pp[:, 7]
            b1_sb = params.tile([128, FC], f32, tag="pc32")
            nc.sync.dma_start(out=b1_sb, in_=b1[l].rearrange("(c p) -> p c", p=128))
            bv_row = params.tile([1, D], bf16, tag="bv_row")
            nc.sync.dma_start(out=bv_row, in_=bvb[l][None, :])

            xbf = xbf_cur

            # ---- K projection (local keys, feature-major) --------------
            wk_sb = wpool.tile([128, DC, D], bf16, tag="w")
            nc.sync.dma_start(out=wk_sb, in_=wk[l].rearrange("(c p) m -> p c m", p=128))
            kt_sb = ktpool.tile([128, DC, T], bf16, tag="kt")
            for mc in range(DC):
                ps = pmisc.tile([128, 512], f32, tag="pmisc")
                for c in range(DC):
                    nc.tensor.matmul(ps, wk_sb[:, c, ts(mc, 128)], xbf[:, c, :],
                                     start=(c == 0), stop=(c == DC - 1))
                nc.vector.tensor_scalar(kt_sb[:, mc, :], ps,
                                        bk_sb[:, mc:mc + 1], None, OP.add)

            # ---- Q projection ------------------------------------------
            wq_sb = wpool.tile([128, DC, D], bf16, tag="w")
            nc.sync.dma_start(out=wq_sb, in_=wq[l].rearrange("(c p) m -> p c m", p=128))
            qT = qtpool.tile([128, DC, T], bf16, tag="qt")
            for mc in range(DC):
                ps = pmisc.tile([128, 512], f32, tag="pmisc")
                for c in range(DC):
                    nc.tensor.matmul(ps, wq_sb[:, c, ts(mc, 128)], xbf[:, c, :],
                                     start=(c == 0), stop=(c == DC - 1))
                nc.vector.tensor_scalar(qT[:, mc, :], ps,
                                        bq_sb[:, mc:mc + 1], None, OP.add)

            # ---- V projection (token-major, 65-col per head) -----------
            wv_sb = wpool.tile([128, DC, D], bf16, tag="w")
            nc.sync.dma_start(out=wv_sb, in_=wv[l].rearrange("(c p) m -> p c m", p=128))
            v65 = vpool.tile([128, TC, H * 65], bf16, tag="v65")
            for t_ in range(TC):
                v65v = v65[:, t_, :].rearrange("p (h w) -> p h w", w=65)
                nc.vector.memset(v65v[:, :, 64:65], 1.0)
                for nh in range(2):
                    ps = pmisc.tile([128, 512], f32, tag="pmisc")
                    for c in range(DC):
                        nc.tensor.matmul(ps, xbf[:, c, ts(t_, 128)],
                                         wv_sb[:, c, ds(nh * 512, 512)],
                                         start=(c == 0), stop=False)
                    nc.tensor.matmul(ps, ones_row_bf, bv_row[:, ds(nh * 512, 512)],
                                     start=False, stop=True)
                    nc.vector.tensor_copy(
                        v65v[:, ds(nh * 8, 8), 0:64],
                        ps.rearrange("p (h d) -> p h d", d=64))

            # ---- attention vs local KV shard ---------------------------
            att_dA = dram.tile([8 * 65, T], bf16, tag="attdA")
            att_dB = dram.tile([8 * 65, T], bf16, tag="attdB")
            att_gA = dram.tile([8 * 65, T], bf16, tag="attgA")
            att_gB = dram.tile([8 * 65, T], bf16, tag="attgB")
            for j in range(DC):  # head pairs (2j, 2j+1)
                att_d = att_dA if j < 4 else att_dB
                at_tiles = []
                for kc in range(TC):
                    pss = pscore.tile([128, 1024], f32, tag="pscore")
                    nc.tensor.matmul(pss[:, 0:512], kt_sb[0:64, j, ts(kc, 128)],
                                     qT[0:64, j, :], start=True, stop=True,
                                     tile_position=(0, 0))
                    nc.tensor.matmul(pss[:, 512:1024], kt_sb[64:128, j, ts(kc, 128)],
                                     qT[64:128, j, :], start=True, stop=True,
                                     tile_position=(64, 0))
                    at = apool.tile([128, 1024], bf16, tag="attn")
                    nc.scalar.activation(at, pss, AF.Exp, scale=1.0 / 32.0,
                                         bias=mb_sb[:, kc:kc + 1])
                    at_tiles.append(at)
                for ab in range(2):
                    h = 2 * j + ab
                    pav_t = pav.tile([65, 512], f32, tag="pav")
                    for kc in range(TC):
                        nc.tensor.matmul(pav_t,
                                         v65[:, kc, ds((h % 16) * 65, 65)],
                                         at_tiles[kc][:, ds(ab * 512, 512)],
                                         start=(kc == 0), stop=(kc == TC - 1))
                    ao65 = ao65pool.tile([65, 512], bf16, tag="ao65")
                    nc.vector.tensor_copy(ao65, pav_t)
                    nc.sync.dma_start(out=att_d[ds((h % 8) * 65, 65), :], in_=ao65)
                if j == 3:
                    nc.gpsimd.collective_compute(
                        "AllReduce", OP.add, replica_groups=groups,
                        ins=[att_dA.opt()], outs=[att_gA.opt()])
            nc.gpsimd.collective_compute(
                "AllReduce", OP.add, replica_groups=groups,
                ins=[att_dB.opt()], outs=[att_gB.opt()])

            # ---- denominators + normalize ------------------------------
            den_sb = denp.tile([16, 512], bf16, tag="den")
            nc.sync.dma_start(
                out=den_sb[0:8, :],
                in_=att_gA.rearrange("(h w) q -> h w q", w=65)[:, 64, :])
            nc.sync.dma_start(
                out=den_sb[8:16, :],
                in_=att_gB.rearrange("(h w) q -> h w q", w=65)[:, 64, :])
            den_r = denp.tile([16, 512], bf16, tag="denr")
            with nc.allow_low_precision(reason="softmax denom reciprocal in bf16"):
                nc.vector.reciprocal(den_r, den_sb)
            denr_d = dram.tile([16, 512], bf16, tag="denr")
            nc.sync.dma_start(out=denr_d[:, :], in_=den_r)

            an_bf = xbfpool.tile([128, DC, T], bf16, tag="xbf")
            for j in range(DC):
                att_g = att_gA if j < 4 else att_gB
                anf = anfpool.tile([128, 512], bf16, tag="anf")
                for ab in range(2):
                    h = 2 * j + ab
                    nc.sync.dma_start(out=anf[ds(ab * 64, 64), :],
                                      in_=att_g[ds((h % 8) * 65, 64), :])
                rb = rbp.tile([128, 512], bf16, tag="rb")
                for ab in range(2):
                    src = bass.AP(tensor=denr_d.tensor,
                                  offset=denr_d.offset + (2 * j + ab) * 512,
                                  ap=[[0, 64], [1, 512]])
                    nc.sync.dma_start(out=rb[ds(ab * 64, 64), :], in_=src)
                nc.vector.tensor_tensor(an_bf[:, j, :], anf, rb, OP.mult)

            # ---- Wo + residual -----------------------------------------
            wo_sb = wpool.tile([128, DC, D], bf16, tag="w")
            nc.sync.dma_start(out=wo_sb, in_=wo[l].rearrange("(c p) m -> p c m", p=128))
            for mc in range(DC):
                ps = pmisc.tile([128, 512], f32, tag="pmisc")
                for c in range(DC):
                    nc.tensor.matmul(ps, wo_sb[:, c, ts(mc, 128)], an_bf[:, c, :],
                                     start=(c == 0), stop=(c == DC - 1))
                nc.vector.scalar_tensor_tensor(xT[:, mc, :], ps,
                                               bo_sb[:, mc:mc + 1], xT[:, mc, :],
                                               OP.add, OP.add)

            # ---- LN1 ----------------------------------------------------
            xbf2 = xbfpool.tile([128, DC, T], bf16, tag="xbf")
            layernorm_inplace(g1_sb, be1_sb, 1, xbf2)

            # ---- FFN ----------------------------------------------------
            ffT = vpool.tile([128, FC, T], bf16, tag="fft", bufs=1)
            for b4 in range(4):
                w1b = wpool.tile([128, DC, 1024], bf16, tag="w")
                nc.sync.dma_start(
                    out=w1b,
                    in_=w1[l][:, ds(b4 * 1024, 1024)].rearrange("(c p) m -> p c m", p=128))
                for mcb in range(DC):
                    mc = b4 * 8 + mcb
                    ps = pmisc.tile([128, 512], f32, tag="pmisc")
                    for c in range(DC):
                        nc.tensor.matmul(ps, w1b[:, c, ts(mcb, 128)], xbf2[:, c, :],
                                         start=(c == 0), stop=(c == DC - 1))
                    nc.vector.tensor_scalar(ffT[:, mc, :], ps,
                                            b1_sb[:, mc:mc + 1], 0.0,
                                            OP.add, OP.max)
            for mc in range(DC):
                w2mc = w2pool.tile([128, FC, 128], bf16, tag="w2")
                nc.sync.dma_start(out=w2mc,
                                  in_=w2p[l, mc].rearrange("o p m -> p o m"))
                ps = pmisc.tile([128, 512], f32, tag="pmisc")
                for fc in range(FC):
                    nc.tensor.matmul(ps, w2mc[:, fc, :], ffT[:, fc, :],
                                     start=(fc == 0), stop=(fc == FC - 1))
                nc.vector.scalar_tensor_tensor(xT[:, mc, :], ps,
                                               b2_sb[:, mc:mc + 1], xT[:, mc, :],
                                               OP.add, OP.add)

            # ---- LN2 ----------------------------------------------------
            xbf_cur = xbfpool.tile([128, DC, T], bf16, tag="xbf")
            layernorm_inplace(g2_sb, be2_sb, 2, xbf_cur)

        # ---- output ----------------------------------------------------
        nc.sync.dma_start(out=outT[:, :].rearrange("(c p) t -> p c t", p=128), in_=xT)
        ctx.close()

    nc.compile()
    return nc


def _prepare_host(inputs):
    src = np.asarray(inputs["src"]).astype(np.int64)
    emb = np.asarray(inputs["emb"], dtype=np.float32)
    x = emb[src]                                    # [B, S, D] f32
    pos = np.arange(B, dtype=np.float32)[:, None]
    div = np.exp(np.arange(0, D, 2, dtype=np.float32) * (-np.log(10000.0) / D))
    pe = np.zeros((B, D), np.float32)
    pe[:, 0::2] = np.sin(pos / div)
    pe[:, 1::2] = np.cos(pos / div)
    x = x + pe[:, None, :]

    mask = np.asarray(inputs["src_mask"]).reshape(B, S)
    mbias = np.where(mask != 0, 0.0, -1e9).astype(np.float32)   # [B, S]

    f = np.float32
    shared = {
        "wq": np.ascontiguousarray(np.asarray(inputs["Wq"], f).astype(BF16)),
        "wk": np.ascontiguousarray(np.asarray(inputs["Wk"], f).astype(BF16)),
        "wv": np.ascontiguousarray(np.asarray(inputs["Wv"], f).astype(BF16)),
        "wo": np.ascontiguousarray(np.asarray(inputs["Wo"], f).astype(BF16)),
        "w1": np.ascontiguousarray(np.asarray(inputs["W1"], f).astype(BF16)),
        "w2p": np.ascontiguousarray(
            np.asarray(inputs["W2"], f).reshape(L, FC, 128, DC, 128)
            .transpose(0, 3, 1, 2, 4).astype(BF16)),
        "bq": np.ascontiguousarray(np.asarray(inputs["bq"], f)),
        "bk": np.ascontiguousarray(np.asarray(inputs["bk"], f)),
        "bvb": np.ascontiguousarray(np.asarray(inputs["bv"], f).astype(BF16)),
        "bo": np.ascontiguousarray(np.asarray(inputs["bo"], f)),
        "b1": np.ascontiguousarray(np.asarray(inputs["b1"], f)),
        "b2": np.ascontiguousarray(np.asarray(inputs["b2"], f)),
        "g1": np.ascontiguousarray(np.asarray(inputs["g1"], f)),
        "be1": np.ascontiguousarray(np.asarray(inputs["be1"], f)),
        "g2": np.ascontiguousarray(np.asarray(inputs["g2"], f)),
        "be2": np.ascontiguousarray(np.asarray(inputs["be2"], f)),
    }
    in_maps = []
    for i in range(NCORES):
        b = i // R
        t0 = (i % R) * T
        m = dict(shared)
        m["x0T"] = np.ascontiguousarray(x[b, t0:t0 + T, :].T.astype(np.float32))
        m["maskb"] = np.ascontiguousarray(
            mbias[b, t0:t0 + T].reshape(TC, 128).T)
        in_maps.append(m)
    return in_maps


def _run(in_maps, trace=False):
    from concourse.bass_utils import run_bass_kernel_spmd
    if "nc" not in _CACHE:
        _CACHE["nc"] = _build_nc()
    nc = _CACHE["nc"]
    res = run_bass_kernel_spmd(nc, in_maps, core_ids=list(range(NCORES)),
                               trace=trace)
    outs = res.results
    y = np.zeros((B, S, D), np.float32)
    for i in range(NCORES):
        b = i // R
        t0 = (i % R) * T
        y[b, t0:t0 + T, :] = outs[i]["outT"].T
    return y, res


def kernel(**inputs) -> np.ndarray:
    in_maps = _prepare_host(inputs)
    y, _ = _run(in_maps, trace=False)
    return y


def kernel_traced(**inputs):
    """Same as kernel() but returns (output, BassKernelResults with profile)."""
    in_maps = _prepare_host(inputs)
    return _run(in_maps, trace=True)
